# revision 50
# baseline (speedup 1.0000x reference)
"""Trainium2 Bass kernel for masked causal attention with RoPE (mgdt column masking).

Reference computation (B=4, T=2048, H=512, heads=8, D=64):
  q/k/v = x @ W + b;  RoPE(q, k) over full hidden dim (pairs of adjacent channels);
  scores = q k^T / sqrt(D) with causal tril mask plus fully-masked key columns
  at {4, 7, 10, ...} (period 3); softmax; out = (att @ v) @ Wo + bo.

Sharding: 8 cores = data-parallel over batch (4) x tensor-parallel over head
groups (2 x 4 heads). Each core computes a [T, H] partial of its batch's
output projection (Wo row-sharded); host sums the pair of partials + bo.

Structure (184us baseline -> 128us):
  - Rope combine: ACT (idle pre-attention) drains the projection PSUM to
    bf16 SBUF first, so the rope STT/add ops run in the DVE 2x/4x all-SBUF
    perf mode instead of 1x fp32-PSUM mode, and each PSUM slab turns over
    after one ACT op instead of two DVE reads.
  - The normalize epilogue is STAGED across tile slots (den-row extract at
    +2, broadcast/reciprocal/scale at +4, out-projections at +6,+8,..) so
    its PE-queue entries always arrive with their DVE dependencies already
    met and never head-of-line block the score/exp stream.
  - Q/K projections run in fp8e4m3 with DoubleRow perf mode (2 matmuls of
    K=256 instead of 4 of K=128): weights are prescaled x16 on the host to
    escape fp8 subnormals, with 1/16 folded into the rope trig tables and
    biases x16. Scores tolerate fp8 (error /8 then softmax-normalized); the
    V path stays bf16 because quantization error in a random-sign dot
    product does NOT average down (error grows with the same sqrt(N) as the
    signal) and V feeds the output directly — all-fp8 measured 2.2e-2 L2,
    fp8-QK-only measures 6.5e-3.
  - KEY COMPACTION: the 682 fully-masked key columns are removed on the host
    (1366 kept, padded to 1408); causal mask becomes per-(s_tile, t_chunk)
    band masks, duplicated host-side for both heads of a pair so the mask
    multiply is a plain all-bf16 DVE op (2x/4x perf mode).
  - Consumption-ordered DMA: inputs are block-packed per consumption unit
    (weights, x row-tiles per chunk, trig tables per kc/chunk block) and
    issued in the order the dense projection phase consumes them.
  - Phase A: all q/k/v projections, K/V interleaved, psum double-buffered.
    (Weaving projections INTO the attention stream was tried twice and
    regresses: with 2 score slabs each insert stalls the exp pipeline.)
  - Attention is SOFTWARE-PIPELINED: attv(k) is emitted after scores(k+1),
    and unit epilogues / out-projections are deferred ~2 tile-slots into the
    following unit's stream, so the PE queue never head-of-line blocks the
    score->exp stream. scores pack 2 heads via PE row-tiling (concurrent
    K=64 matmuls at base partitions 0/64); exp once per [128, 1024] pair;
    V augmented with a ones column so att@V emits softmax denominators free.
  - DMA-free normalization (old: 5 DMAs + fp32 selector matmul per unit, all
    head-of-line on one queue): den row -> f16 [1,1024] cast; two col-tiled
    K=1 f16 matmuls broadcast it across partitions; reciprocal_approx_fast
    on the [128,512] broadcast (a [1,1024] DVE reciprocal costs 6.5us!);
    DVE muls read the po PSUM against the SBUF reciprocals directly.
  - ACT queue: the exp stream + phase-A vaug copies + late-chunk output
    copies (after exps end). GpSimd: only output-DMA issue (no PSUM port).
  - Output is written bf16 (halves output DMA); host sums partials in fp32.
  PSUM budget (8 banks): scores [128,1024]x2 + po [65,1024]x2; prd/pout/proj
  ride the scores pool's rotation.
"""

import sys

if "/opt/trn_rl_repo" not in sys.path:
    sys.path.insert(0, "/opt/trn_rl_repo")

import numpy as np
import ml_dtypes

B, T, H, NH, D = 4, 2048, 512, 8, 64
THETA = 10000.0
PERIOD, RET_ORDER = 3, 2
NCORES = 8
CPG = H // 2          # 256 channels per head-group shard
CHUNK = 512           # t-chunk (one PSUM bank of fp32)
NCH = T // CHUNK      # 4 query chunks
BF = ml_dtypes.bfloat16
F16 = np.float16

# --- compacted key geometry (host + builder share this) ---
_cm = np.ones(T, bool)
_cm[PERIOD + RET_ORDER - 1::PERIOD] = False
POS = np.where(_cm)[0]              # 1366 unmasked key positions
NSC_RAW = len(POS)                  # 1366
NSTC = (NSC_RAW + 127) // 128       # 11 s-tiles
NSC = NSTC * 128                    # 1408 padded
KCW = [CHUNK, CHUNK, NSC - 2 * CHUNK]   # k-projection block widths (512,512,384)
XB = [0, 4 * KCW[0], 4 * (KCW[0] + KCW[1])]   # xtc2 block bases
TKB = XB                                       # trigk block bases (same widths)

# per s-tile first/last valid original positions
_INF = 1 << 30
TILE_LO = [int(POS[128 * i]) if 128 * i < NSC_RAW else _INF for i in range(NSTC)]
TILE_HI = [int(POS[min(128 * i + 127, NSC_RAW - 1)]) if 128 * i < NSC_RAW else _INF
           for i in range(NSTC)]


def _tiles_for_chunk(j):
    """(i, col0, crossing) for each compact s-tile contributing to t-chunk j."""
    out = []
    for i in range(NSTC):
        lo, hi = TILE_LO[i], TILE_HI[i]
        if lo > CHUNK * j + CHUNK - 1:
            continue
        col0 = max(0, lo - CHUNK * j)
        crossing = hi > CHUNK * j  # some (row, col) pairs invalid -> needs mask
        out.append((i, col0, crossing))
    return out


# crossings in usage order (j descending, tile ascending) — also the DMA order
_CROSS_USE = [(i, j) for j in (3, 2, 1, 0)
              for (i, c0, cr) in _tiles_for_chunk(j) if cr]
_CROSSINGS = sorted(set(_CROSS_USE))

_prog = None


def _build_program():
    global _prog
    if _prog is not None:
        return _prog
    from contextlib import ExitStack
    import concourse.bacc as bacc
    import concourse.tile as tile
    from concourse import mybir

    bf = mybir.dt.bfloat16
    f16 = mybir.dt.float16
    f32 = mybir.dt.float32
    f8 = mybir.dt.float8e4
    DR = mybir.MatmulPerfMode.DoubleRow
    EXP = mybir.ActivationFunctionType.Exp
    COPY = mybir.ActivationFunctionType.Copy
    ADD = mybir.AluOpType.add
    MULT = mybir.AluOpType.mult
    import concourse.bass as _bass

    def pair_ap(base, stride, n):
        """[[part],[stride,2],[1,n]] 3D AP for a DoubleRow kt-pair."""
        return _bass.AP(tensor=base.tensor, offset=base.offset,
                        ap=[base.ap[0], [stride, 2], [1, n]])

    nc = bacc.Bacc("TRN2", target_bir_lowering=False, debug=False, num_devices=NCORES)

    def din(name, shape, dt):
        return nc.dram_tensor(name, shape, dt, kind="ExternalInput").ap()

    nm = len(_CROSSINGS)
    xtc2_d = din("xtc2", [128, 4 * NSC], f8)      # block-packed compacted x^T
    xtc2b_d = din("xtc2b", [128, 4 * NSC], bf)    # same, bf16 (V path)
    xt2_d = din("xt2", [128, 4 * T], f8)          # block-packed x^T (per chunk)
    xtc1_d = din("xtc1", [1, NSC], bf)            # ones row (zero at pad cols)
    wq4_d = din("wq4", [128, 2048], f8)           # wq|wqs x16, each 4 kt x 256
    wk4_d = din("wk4", [128, 2048], f8)           # wk|wks x16
    wv_d = din("wv", [128, 4 * 260], bf)          # Wv (bf16: feeds output)
    wvb_d = din("wvb", [1, 260], bf)
    wo_d = din("wo", [128, 2 * H], bf)
    trigk_d = din("trigk", [128, 4 * NSC], bf)    # per-kc blocks cosk/sink x 2ct
    trigq_d = din("trigq", [128, 4 * T], bf)      # per-j blocks cos/sin x 2ct
    bm_d = din("bmask", [128, nm * 2 * CHUNK], bf)  # masks duplicated per head
    bias_d = din("biases", [128, 8], f32)         # bq|bqs|bk|bks (2 cols each)
    out_d = nc.dram_tensor("out", [T, H], bf, kind="ExternalOutput").ap()

    tiles_by_j = {j: _tiles_for_chunk(j) for j in range(NCH)}

    with tile.TileContext(nc) as tc:
        with ExitStack() as ctx:
            sg = ctx.enter_context(tc.tile_pool(name="sg", bufs=1))

            # ---------- persistent SBUF tiles ----------
            xtc2 = sg.tile([128, 4 * NSC], f8, tag="xtc2", name="xtc2")
            xtc2b = sg.tile([128, 4 * NSC], bf, tag="xtc2b", name="xtc2b")
            xt2 = sg.tile([128, 4 * T], f8, tag="xt2", name="xt2")
            xtc1 = sg.tile([1, NSC], bf, tag="xtc1", name="xtc1")
            wq4 = sg.tile([128, 2048], f8, tag="wq4", name="wq4")
            wk4 = sg.tile([128, 2048], f8, tag="wk4", name="wk4")
            wv = sg.tile([128, 4 * 260], bf, tag="wv", name="wv")
            wvb = sg.tile([1, 260], bf, tag="wvb", name="wvb")
            wo = sg.tile([128, 2 * H], bf, tag="wo", name="wo")
            trigk = sg.tile([128, 4 * NSC], bf, tag="trigk", name="trigk")
            trigq = sg.tile([128, 4 * T], bf, tag="trigq", name="trigq")
            bm = sg.tile([128, nm * 2 * CHUNK], bf, tag="bm", name="bm")
            bias_sb = sg.tile([128, 8], f32, tag="biases", name="bias_sb")
            ones64 = sg.tile([1, 64], f16, tag="ones64", name="ones64")
            warm_in = sg.tile([1, 8], f32, tag="warmi", name="warm_in")
            warm_out = sg.tile([1, 8], f32, tag="warmo", name="warm_out")

            qrot, krot, aot = {}, {}, {}
            for ct in range(2):
                for j in range(NCH):
                    qrot[ct, j] = sg.tile([128, CHUNK], bf, tag=f"qr{ct}_{j}",
                                          name=f"qr{ct}_{j}")
                    aot[ct, j] = sg.tile([128, CHUNK], bf, tag=f"ao{ct}_{j}",
                                         name=f"ao{ct}_{j}")
                for kc in range(3):
                    krot[ct, kc] = sg.tile([128, CHUNK], bf, tag=f"kr{ct}_{kc}",
                                           name=f"kr{ct}_{kc}")
            vaug = [sg.tile([128, 260], bf, tag=f"va{s}", name=f"va{s}")
                    for s in range(NSTC)]

            # ---------- input DMA, consumption order, on the SP queue ----------
            def feed(dst, src, c0, c1):
                nc.sync.dma_start(out=dst[:, c0:c1], in_=src[:, c0:c1])

            nc.sync.dma_start(out=bias_sb, in_=bias_d[:, :])
            nc.sync.dma_start(out=xtc1, in_=xtc1_d[:, :])
            nc.sync.dma_start(out=wvb, in_=wvb_d[:, :])
            def bmn(i, j):
                return _CROSSINGS.index((i, j))

            def bm_feed(jwant):
                for (i, j) in _CROSS_USE:
                    if j == jwant:
                        n = bmn(i, j)
                        feed(bm, bm_d, n * 2 * CHUNK, (n + 1) * 2 * CHUNK)

            feed(wk4, wk4_d, 0, 2048)
            feed(trigk, trigk_d, TKB[0], TKB[1])          # kc0 block
            feed(xtc2, xtc2_d, XB[0], XB[1])              # b0 block
            feed(wv, wv_d, 0, 4 * 260)
            feed(xtc2b, xtc2b_d, XB[0], XB[1])
            feed(trigk, trigk_d, TKB[1], TKB[2])          # kc1
            feed(xtc2, xtc2_d, XB[1], XB[2])              # b1
            feed(xtc2b, xtc2b_d, XB[1], XB[2])
            feed(trigk, trigk_d, TKB[2], 4 * NSC)         # kc2
            feed(xtc2, xtc2_d, XB[2], 4 * NSC)            # b2
            feed(xtc2b, xtc2b_d, XB[2], 4 * NSC)
            feed(wq4, wq4_d, 0, 2048)
            for j in (3, 2, 1, 0):
                feed(xt2, xt2_d, j * 2048, (j + 1) * 2048)
                feed(trigq, trigq_d, j * 2048, (j + 1) * 2048)
            feed(wo, wo_d, 0, 2 * H)
            for jj in (3, 2, 1, 0):
                bm_feed(jj)

            # ---------- constants + ACT exp-table preload ----------
            nc.gpsimd.memset(ones64, 1.0)
            nc.gpsimd.memset(warm_in, 0.0)
            nc.scalar.activation(out=warm_out, in_=warm_in, func=EXP, scale=0.125)

            # ---------- pools ----------
            rtmp = ctx.enter_context(tc.tile_pool(name="rtmp", bufs=6))
            ptp = ctx.enter_context(tc.tile_pool(name="ptp", bufs=3))
            dn = ctx.enter_context(tc.tile_pool(name="dn", bufs=2))
            ost = ctx.enter_context(tc.tile_pool(name="ost", bufs=4))
            psp = ctx.enter_context(tc.tile_pool(name="psp", bufs=2, space="PSUM"))
            pop = ctx.enter_context(tc.tile_pool(name="pop", bufs=2, space="PSUM"))

            # ---------- projection / attention building blocks ----------
            def wslc(wt, v, kt, ct):
                base = 1024 * v + 256 * kt + 128 * ct
                return wt[:, base:base + 128]

            def project_rope(wt, ct, bcol, dst, xsl, c_off, s_off, w, tag):
                """dst <- rope(x @ W + b) for one chunk of width w."""
                ps = psp.tile([128, 2 * CHUNK], f32, tag="ps", name=f"pj_{tag}")
                pm = ps[:, 0:w]
                pms = ps[:, CHUNK:CHUNK + w]
                for v, dstp in ((0, pm), (1, pms)):
                    for p in range(2):   # kt-pairs (0,1), (2,3) via DoubleRow
                        lw = pair_ap(wt[:, 1024 * v + 512 * p + 128 * ct:],
                                     256, 128)
                        nc.tensor.matmul(dstp, lhsT=lw, rhs=xsl(p),
                                         start=(p == 0), stop=(p == 1),
                                         perf_mode=DR)
                # ACT (idle pre-attention) drains PSUM to bf16 SBUF so the
                # rope STTs run in the DVE 2x/4x all-SBUF-bf16 perf mode
                # (an fp32-PSUM input locks them to 1x) and the PSUM slab
                # turns over after one ACT op instead of two DVE reads.
                pc = rtmp.tile([128, 2 * CHUNK], bf, tag="pc", name=f"pc_{tag}")
                nc.scalar.copy(out=pc[:, 0:CHUNK + w], in_=ps[:, 0:CHUNK + w])
                t1 = rtmp.tile([128, CHUNK], bf, tag="t1", name=f"t1_{tag}")
                nc.vector.scalar_tensor_tensor(
                    out=t1[:, :w], in0=pc[:, 0:w],
                    scalar=bias_sb[:, bcol:bcol + 1],
                    in1=trig_at(c_off, w), op0=ADD, op1=MULT)
                t2 = rtmp.tile([128, CHUNK], bf, tag="t2", name=f"t2_{tag}")
                nc.vector.scalar_tensor_tensor(
                    out=t2[:, :w], in0=pc[:, CHUNK:CHUNK + w],
                    scalar=bias_sb[:, bcol + 1:bcol + 2],
                    in1=trig_at(s_off, w), op0=ADD, op1=MULT)
                nc.vector.tensor_add(dst[:, :w], t1[:, :w], t2[:, :w])

            # trig offsets: q offsets carry a flag bit so trig_at picks the table
            def k_cos(ct, kc):
                return TKB[kc] + KCW[kc] * (2 * ct)

            def k_sin(ct, kc):
                return TKB[kc] + KCW[kc] * (2 * ct + 1)

            def q_cos(ct, j):
                return (1 << 20) + 2048 * j + 512 * (2 * ct)

            def q_sin(ct, j):
                return (1 << 20) + 2048 * j + 512 * (2 * ct + 1)

            def trig_at(off, w):
                if off >= (1 << 20):
                    return trigq[:, off - (1 << 20):off - (1 << 20) + w]
                return trigk[:, off:off + w]

            def proj_k(ct, kc):
                w = KCW[kc]
                project_rope(wk4, ct, 4 + 2 * ct, krot[ct, kc],
                             lambda p: pair_ap(xtc2[:, XB[kc] + 2 * w * p:], w, w),
                             k_cos(ct, kc), k_sin(ct, kc), w, f"k{ct}_{kc}")

            def proj_q(ct, j):
                project_rope(wq4, ct, 2 * ct, qrot[ct, j],
                             lambda p: pair_ap(xt2[:, 2048 * j + 1024 * p:],
                                               512, 512),
                             q_cos(ct, j), q_sin(ct, j), CHUNK, f"q{ct}_{j}")

            def proj_v(s, on_act=False):
                # V stays bf16: its values feed the output directly, and fp8
                # quantization does NOT average down in a random-sign dot
                # product (error grows with the same sqrt(N) as the signal).
                ps = psp.tile([128, 2 * CHUNK], f32, tag="ps", name=f"pv{s}")
                pv = ps[:, 0:260]
                ssl = slice(128 * s, 128 * (s + 1))
                b = s // 4
                wb = KCW[b]
                for kt in range(4):
                    o = XB[b] + wb * kt + 128 * (s % 4)
                    nc.tensor.matmul(pv, lhsT=xtc2b[:, o:o + 128],
                                     rhs=wv[:, 260 * kt:260 * (kt + 1)],
                                     start=(kt == 0), stop=False)
                nc.tensor.matmul(pv, lhsT=xtc1[0:1, ssl], rhs=wvb,
                                 start=False, stop=True)
                if on_act:          # ACT is idle before the exp stream starts
                    nc.scalar.copy(out=vaug[s], in_=pv)
                else:
                    nc.vector.tensor_copy(out=vaug[s], in_=pv)

            # ---------- attention ----------
            def att_scores(j, hp, s, col0):
                """scores pair + exp + mask; returns the pt tile."""
                ct = hp
                ksl = slice(128 * (s % 4), 128 * (s % 4) + 128)
                ps = psp.tile([128, 2 * CHUNK], f32, tag="ps",
                              name=f"ps{j}_{hp}_{s}")
                for idx in range(2):
                    pb = 64 * idx
                    nc.tensor.matmul(
                        ps[:, CHUNK * idx + col0:CHUNK * (idx + 1)],
                        lhsT=krot[ct, s // 4][pb:pb + 64, ksl],
                        rhs=qrot[ct, j][pb:pb + 64, col0:],
                        start=True, stop=True)
                pt = ptp.tile([128, 2 * CHUNK], bf, tag="pt",
                              name=f"pt{j}_{hp}_{s}")
                nc.scalar.activation(out=pt[:, col0:], in_=ps[:, col0:],
                                     func=EXP, scale=0.125)
                if (s, j) in set(_CROSSINGS):
                    n = bmn(s, j)
                    nc.vector.tensor_mul(
                        pt[:, col0:], pt[:, col0:],
                        bm[:, n * 2 * CHUNK + col0:(n + 1) * 2 * CHUNK])
                return pt

            def att_attv(hp, s, col0, first, last, po, pt):
                for idx in range(2):
                    hh = 2 * hp + idx
                    nc.tensor.matmul(
                        po[:, CHUNK * idx + col0:CHUNK * (idx + 1)],
                        lhsT=vaug[s][:, 65 * hh:65 * hh + 65],
                        rhs=pt[:, CHUNK * idx + col0:CHUNK * (idx + 1)],
                        start=first, stop=last,
                        skip_group_check=True)

            def epi_a(j, hp, po):
                """Epilogue stage 1: pull the denominator row out of PSUM."""
                densb = dn.tile([1, 2 * CHUNK], f16, tag="densb",
                                name=f"dn{j}_{hp}")
                nc.vector.tensor_copy(out=densb, in_=po[64:65, :])
                return densb

            def epi_b(j, hp, po, densb):
                """Epilogue stage 2: broadcast + reciprocal + scale. Emitted a
                slot after epi_a so its matmuls reach the head of the PE queue
                with the densb dependency already satisfied."""
                ct = hp
                prd = psp.tile([128, 2 * CHUNK], f32, tag="ps",
                               name=f"prd{j}_{hp}")
                nc.tensor.matmul(prd[0:64, 0:CHUNK], lhsT=ones64,
                                 rhs=densb[0:1, 0:CHUNK], start=True, stop=True)
                nc.tensor.matmul(prd[64:128, 0:CHUNK], lhsT=ones64,
                                 rhs=densb[0:1, CHUNK:2 * CHUNK],
                                 start=True, stop=True)
                prdsb = dn.tile([128, CHUNK], f32, tag="prdsb",
                                name=f"pr{j}_{hp}")
                nc.vector.reciprocal_approx_fast(out=prdsb, in_=prd[:, 0:CHUNK])
                nc.vector.tensor_mul(aot[ct, j][0:64, :], po[0:64, 0:CHUNK],
                                     prdsb[0:64, :])
                nc.vector.tensor_mul(aot[ct, j][64:128, :],
                                     po[0:64, CHUNK:2 * CHUNK],
                                     prdsb[64:128, :])

            def att_unit_epilogue(j, hp, po):
                epi_b(j, hp, po, epi_a(j, hp, po))

            def out_proj_tt(j, tts):
                for tt in tts:
                    pout = psp.tile([128, 2 * CHUNK], f32, tag="ps",
                                    name=f"po{j}_{tt}")
                    for ct2 in range(2):
                        nc.tensor.matmul(
                            pout[:, 0:H],
                            lhsT=aot[ct2, j][:, 128 * tt:128 * (tt + 1)],
                            rhs=wo[:, H * ct2:H * (ct2 + 1)],
                            start=(ct2 == 0), stop=(ct2 == 1))
                    osb = ost.tile([128, H], bf, tag="ost", name=f"ob{j}_{tt}")
                    if j <= 1:      # ACT's exp stream is done/ending by then
                        nc.scalar.copy(out=osb, in_=pout[:, 0:H])
                    else:
                        nc.vector.tensor_copy(out=osb, in_=pout[:, 0:H])
                    nc.gpsimd.dma_start(
                        out=out_d[CHUNK * j + 128 * tt:CHUNK * j + 128 * (tt + 1), :],
                        in_=osb)

            # ---------- schedule ----------
            # phase A: all projections, interleaved K/V/Q (DMA-overlapped)
            for ct in range(2):
                proj_k(ct, 0)
            for s in range(4):
                proj_v(s, on_act=True)
            for ct in range(2):
                proj_k(ct, 1)
            for s in range(4, 8):
                proj_v(s, on_act=True)
            for ct in range(2):
                proj_k(ct, 2)
            for s in range(8, NSTC):
                proj_v(s, on_act=True)
            for j in (3, 2, 1, 0):
                for ct in range(2):
                    proj_q(ct, j)

            # attention, software-pipelined: attv(k) is emitted AFTER
            # scores(k+1) so the PE queue never blocks the score/exp stream
            # on exp/mask completion; unit epilogues and out-projections are
            # deferred into later tile slots the same way.
            units = [(j, hp) for j in (3, 2, 1, 0) for hp in range(2)]
            po_of, seq, last_k = {}, [], {}
            for u, (j, hp) in enumerate(units):
                tiles_j = tiles_by_j[j]
                for si, (s, col0, crossing) in enumerate(tiles_j):
                    seq.append((u, j, hp, s, col0,
                                si == 0, si == len(tiles_j) - 1))
                last_k[u] = len(seq) - 1

            # deferred actions: slot k -> [closure]
            defer = defaultdict_list = {}

            def at(k, fn):
                defer.setdefault(k, []).append(fn)

            densb_of = {}

            def stage_a(j, hp):
                densb_of[(j, hp)] = epi_a(j, hp, po_of[(j, hp)])

            def stage_b(j, hp):
                epi_b(j, hp, po_of[(j, hp)], densb_of[(j, hp)])

            for u, (j, hp) in enumerate(units):
                at(last_k[u] + 2, lambda j=j, hp=hp: stage_a(j, hp))
                at(last_k[u] + 4, lambda j=j, hp=hp: stage_b(j, hp))
                if hp == 1:  # chunk done -> out-projection, one tt per slot
                    for tt in range(4):
                        at(last_k[u] + 6 + 2 * tt,
                           lambda j=j, tt=tt: out_proj_tt(j, (tt,)))

            pts = {}
            for k, (u, j, hp, s, col0, first, last) in enumerate(seq):
                if first:
                    po_of[(j, hp)] = pop.tile([65, 2 * CHUNK], f32, tag="po",
                                              name=f"poacc{j}_{hp}")
                pts[k] = att_scores(j, hp, s, col0)
                if k >= 1:
                    (u2, j2, hp2, s2, c2, f2, l2) = seq[k - 1]
                    att_attv(hp2, s2, c2, f2, l2, po_of[(j2, hp2)], pts.pop(k - 1))
                for fn in defer.pop(k, ()):
                    fn()
            (u2, j2, hp2, s2, c2, f2, l2) = seq[-1]
            att_attv(hp2, s2, c2, f2, l2, po_of[(j2, hp2)], pts.pop(len(seq) - 1))
            for k in sorted(defer):
                for fn in defer.pop(k, ()):
                    fn()

    nc.compile()
    _prog = nc
    return nc


def _host_inputs(x, Wq, bq, Wk, bk, Wv, bv, Wo, bo):
    """Build the 8 per-core input maps (block-packed tensors, hardcoded shapes)."""
    x = np.asarray(x, np.float32)
    Wq, bq = np.asarray(Wq, np.float32), np.asarray(bq, np.float32)
    Wk, bk = np.asarray(Wk, np.float32), np.asarray(bk, np.float32)
    Wv, bv = np.asarray(Wv, np.float32), np.asarray(bv, np.float32)
    Wo = np.asarray(Wo, np.float32)

    def rowpack(a, cols):
        """[R*128, cols] -> [128, R*cols] row-tiles side by side."""
        r = a.shape[0] // 128
        return np.concatenate([a[128 * i:128 * (i + 1)] for i in range(r)], axis=1)

    def blockpack(a, widths):
        """[512, C] -> [128, 4*C] packed per column-block, kt-major inside."""
        cols, c0 = [], 0
        for w in widths:
            for kt in range(4):
                cols.append(a[128 * kt:128 * (kt + 1), c0:c0 + w])
            c0 += w
        return np.concatenate(cols, axis=1)

    FP8 = ml_dtypes.float8_e4m3
    WS = 16.0    # weight prescale: keeps fp8e4m3 weights out of subnormals

    xt_all, xtc_all = [], []
    for b in range(B):
        xt = np.ascontiguousarray(x[b].T)            # (512, 2048)
        xtc = np.zeros((H, NSC), np.float32)
        xtc[:, :NSC_RAW] = xt[:, POS]
        xt_all.append(blockpack(xt, [CHUNK] * NCH).astype(FP8))
        xtcp = blockpack(xtc, KCW)
        xtc_all.append((xtcp.astype(FP8), xtcp.astype(BF)))
    ones_c = np.zeros((1, NSC), np.float32)
    ones_c[0, :NSC_RAW] = 1.0
    xtc1 = ones_c.astype(BF)

    # rope tables (match reference fp32 math)
    inv = (1.0 / (THETA ** (np.arange(0, H, 2, dtype=np.float32) / H))).astype(np.float32)
    tpos = np.arange(T, dtype=np.float32)
    ang = tpos[:, None] * inv[None, :]
    cosf = np.cos(ang).astype(np.float32).T     # (256, T)
    sinf = np.sin(ang).astype(np.float32).T

    swap = np.arange(CPG)
    swap = swap + 1 - 2 * (swap % 2)

    per_g = []
    for g in range(2):
        cols = slice(CPG * g, CPG * (g + 1))
        wq_g, wk_g = Wq[:, cols], Wk[:, cols]
        wv_a = np.zeros((H, 260), np.float32)
        wv_row = np.zeros((1, 260), np.float32)
        for hh in range(4):
            wv_a[:, 65 * hh:65 * hh + 64] = Wv[:, CPG * g + 64 * hh:CPG * g + 64 * (hh + 1)]
            wv_row[0, 65 * hh:65 * hh + 64] = bv[CPG * g + 64 * hh:CPG * g + 64 * (hh + 1)]
            wv_row[0, 65 * hh + 64] = 1.0
        wq2 = np.concatenate([rowpack(WS * w, CPG) for w in (wq_g, wq_g[:, swap])], axis=1)
        wk2 = np.concatenate([rowpack(WS * w, CPG) for w in (wk_g, wk_g[:, swap])], axis=1)

        # trig tables per head-group: rows = the group's 128-channel tiles.
        # The 1/WS undoes the fp8 weight prescale of the q/k projections.
        cos_g = np.repeat(cosf[128 * g:128 * (g + 1)], 2, axis=0) / WS  # (256, T)
        sin_g = np.repeat(sinf[128 * g:128 * (g + 1)], 2, axis=0).copy() / WS
        sin_g[0::2] *= -1.0
        cosk_g = np.zeros((256, NSC), np.float32)
        sink_g = np.zeros((256, NSC), np.float32)
        cosk_g[:, :NSC_RAW] = cos_g[:, POS]
        sink_g[:, :NSC_RAW] = sin_g[:, POS]

        # trigk: per-kc blocks [cosk_ct0 | sink_ct0 | cosk_ct1 | sink_ct1]
        tk, c0 = [], 0
        for w in KCW:
            for ct in range(2):
                tk.append(cosk_g[128 * ct:128 * (ct + 1), c0:c0 + w])
                tk.append(sink_g[128 * ct:128 * (ct + 1), c0:c0 + w])
            c0 += w
        trigk = np.concatenate(tk, axis=1)
        # trigq: per-j blocks [cos_ct0 | sin_ct0 | cos_ct1 | sin_ct1]
        tq = []
        for j in range(NCH):
            sl = slice(CHUNK * j, CHUNK * (j + 1))
            for ct in range(2):
                tq.append(cos_g[128 * ct:128 * (ct + 1), sl])
                tq.append(sin_g[128 * ct:128 * (ct + 1), sl])
        trigq = np.concatenate(tq, axis=1)

        bqs_, bks_ = bq[cols][swap], bk[cols][swap]
        biases = WS * np.stack([
            bq[cols][:128], bqs_[:128], bq[cols][128:], bqs_[128:],
            bk[cols][:128], bks_[:128], bk[cols][128:], bks_[128:],
        ], axis=1).astype(np.float32)
        per_g.append(dict(
            wq4=wq2.astype(FP8), wk4=wk2.astype(FP8),
            wv=rowpack(wv_a, 260).astype(BF), wvb=wv_row.astype(BF),
            wo=rowpack(Wo[cols, :], H).astype(BF),
            trigk=trigk.astype(BF), trigq=trigq.astype(BF), biases=biases,
        ))

    # causal band masks in compacted coords, duplicated for both heads
    spos = np.full(NSC, _INF, np.int64)
    spos[:NSC_RAW] = POS
    bmask = np.zeros((128, len(_CROSSINGS) * 2 * CHUNK), np.float32)
    for n, (i, j) in enumerate(_CROSSINGS):
        rows = spos[128 * i:128 * (i + 1)]
        tcols = np.arange(CHUNK * j, CHUNK * (j + 1))
        m = (rows[:, None] <= tcols[None, :]).astype(np.float32)
        bmask[:, 2 * CHUNK * n:2 * CHUNK * n + CHUNK] = m
        bmask[:, 2 * CHUNK * n + CHUNK:2 * CHUNK * (n + 1)] = m

    shared = dict(bmask=bmask.astype(BF), xtc1=xtc1)
    in_maps = []
    for c in range(NCORES):
        b, g = c // 2, c % 2
        m = dict(xt2=xt_all[b], xtc2=xtc_all[b][0], xtc2b=xtc_all[b][1],
                 **shared)
        m.update(per_g[g])
        in_maps.append(m)
    return in_maps


def run(inputs, trace=False):
    """Build+run; returns BassKernelResults (per-core bf16 partials in .results)."""
    from concourse.bass_utils import run_bass_kernel_spmd
    nc = _build_program()
    in_maps = _host_inputs(**inputs)
    res = run_bass_kernel_spmd(nc, in_maps, list(range(NCORES)), trace=trace)
    return res


def kernel(x, Wq, bq, Wk, bk, Wv, bv, Wo, bo):
    res = run(dict(x=x, Wq=Wq, bq=bq, Wk=Wk, bk=bk, Wv=Wv, bv=bv, Wo=Wo, bo=bo))
    bo = np.asarray(bo, np.float32)
    out = np.empty((B, T, H), np.float32)
    for b in range(B):
        out[b] = (res.results[2 * b]["out"].astype(np.float32)
                  + res.results[2 * b + 1]["out"].astype(np.float32)
                  + bo[None, :])
    return out
